# revision 48
# baseline (speedup 1.0000x reference)
"""2-layer GAT (100000 nodes, 32 neighbors) on 8 trn2 NeuronCores — v3.1.

v3.1 changes over v3 (cost-model timeline: 3245us -> 2483us):
  - Lrelu -> Prelu: AF.Lrelu lives only in the derivative_gelu act-table set,
    so every Lrelu<->Exp switch reloaded the activation table (391 reloads,
    ~500us of Act engine time). AF.Prelu (same math, alpha operand) shares
    the exp_and_others set with Exp AND Copy -> zero reloads.
  - Layer-1 matmuls pack 6 slots into one [128,480] PSUM tile with a single
    PSUM->SBUF copy per group (33 copies -> 6, alternating DVE/Act).
  - table2 rows are padded ids (core*12544+i, 100352 rows) so the x2T_all ->
    table2 chunk mapping never straddles cores; quarter size 25088.
  - The x2->table2 AllGather is split into 4 column parts (25/25/25/23
    A1-tiles) issued inside the A1 loop, hiding ~2/3 of the collective
    under layer-1 compute; T2 loads run-merge across part buffers.
  - T2 packs 2 chunks per [128,288] PSUM tile, copies alternate DVE/Act.

v3.1b (HW-verified, rel err 4.90e-05): the 3 per-tile reciprocals use
reciprocal_approx_fast (single custom-DVE op, ~5x cheaper dispatch than
reciprocal()'s Newton sequence; 294 sites sat on the tile critical chains);
gather sub-batches 12 -> 15 slots (1920 descs < 2048 ring). Both are
model-neutral (critical path is A1 PE.SEQ + A2 DVE) but trim real
sequencer work. NOTE: moving ALL T2 copies to Act regresses the model
-168us (serializes the T2 window) - keep the DVE/Act alternation.

Profiling: no NTFF here; use work/profile_tl.py (TimelineSim + cost model,
single core). Engine busy% there: DVE 76% (bottleneck), DMA 56%, Pool 30%.

Negative results verified this session (do not retry without new evidence):
  - indirect_dma_start with [128,N>1] int32 offset APs returns garbage on
    real ucode (work/probe_indirect.py: not even a permutation of the
    requested rows) despite bass_interp modeling it fine. Only [128,1]
    offsets and the int16 dma_gather path work on HW.
  - nc.gpsimd.tensor_copy reading PSUM (and/or Act Copy with a strided 3D
    out AP) fails neuronx-cc with an opaque CallFunctionObjArgs error.
  - dynamic_dma_scratch_size=65536 overflows SBUF (a2 pool ~114KB/part).
  - Layer-1 restructure (X1 transform -> single AllGather table1 -> int16
    quarter gathers, shared idx16 via inverse-perm values; implemented and
    built OK) REGRESSES the model 2483->3340us: the AG1 barrier kills the
    v3.1 overlap where A2's ~640us gather dispatch + early T2 run UNDER
    the A1 matmul phase (A2 gathers start at ~940us, A1 ends ~1160us).
    Removing A1's 3234 matmuls is worth less than that overlap.
The 33-matmul A1 looks wasteful but is load-bearing for overlap.

BEST NEXT DESIGN (supersedes the AG-pipelined variant; barrier-free):
build table1 by REDUNDANT all-node transform, no collective at all:
  - new input xTf [128, NP] f16 (full x^T, padded-id column order, 25.7MB);
    keep a small xTp [128, SP] (permuted own dests) input.
  - X1-full: 98 groups x 8 chunks (T2's structure): load, matmul vs rhs1
    reordered to [h1|s2|s1] (one contiguous [0:72] copy per chunk), write
    table1 [NP, 128] f16 local-DRAM rows in padded-NATURAL id order.
    ~880 PE instructions vs A1's 3234 (PE.SEQ 1135us -> ~290us).
  - A1g gathers (int16 quarter machinery, elem 72 @ 256B stride) start as
    soon as their table1 QUARTER region is written (range deps; q0 at ~25%
    of X1) - the v3.1 overlap survives WITHOUT an AllGather barrier.
  - s1 layer-1: per-tile 8-col matmul lhsT=xTp[:, t*128:..] rhs=rt1 s1-cols
    -> s1a1 SBUF (no core-dependent addressing).
  - A1g dest tiles are PERMUTED (same perm as A2) so x2T_shard/table2
    become permuted: remap ONLY A2's idx16 values through invp (quarter ==
    core//2 is perm-independent, so counts/sq/mall stay IDENTICAL and are
    shared; layer 1 needs a second idx16 tensor with natural-id values).
  - s1 layer-2 via the xt-matmul trick (lhsT=xt, rhs=rt2[:,0:8] -> s1a2),
    killing the 98 s1d indirects + selfid input.
  - AG2/T2/A2 otherwise unchanged. Est. -250..-350us model; also deletes
    the 106MB xeT host expansion. Needs ~35 min + 1-2 HW cycles.
"""
"""Original v3 notes:

Architecture (vs 10.85 ms baseline, which was SWDGE-descriptor-bound:
33 indirect DMAs x ~1 us Pool fixed cost per 128-node tile, twice):

  Layer 1 — NO gathers: the neighbor lists are static inputs, so the host
  pre-expands transposed node features per edge slot (xeT [128, 98*33*128]
  fp16). Each tile streams its slice and recomputes [h1|s1|s2] per slot with
  33 PE matmuls against the fused rhs1 = [W1 | W1@A1 | W1@A2]. PE is
  otherwise idle; Pool is not involved at all.

  Layer 2 — table2 [100096 rows x 512B stride] fp16 rows [s1(8)|h2(128)|s2(8)]
  built from the AllGathered x2^T, then gathered per destination tile with
  dma_gather (batched SWDGE gather, ~0.34ns/descriptor) in 4 quarter-split
  passes (int16 indices are quarter-local, < 25024). Slots are sorted by
  source within each partition (softmax is order-invariant), padded
  per-quarter to the tile max with dummy index 0, and masked after exp.
  Destinations are bin-packed into tiles (sorted by worst quarter count,
  unified across the 8 cores) to minimize padding; the host un-permutes the
  output rows at the end.

  Attention math runs in gather-natural d-major fp16 layout with contiguous
  DVE ops and a pairwise halving tree (softmax denominator rides in 8 extra
  tree columns).

The Bass program is built on the first kernel() call because tile shapes
depend on the neighbor data (per-tile padded quarter widths).
"""
import sys

if '/opt/trn_rl_repo' not in sys.path:
    sys.path.insert(0, '/opt/trn_rl_repo')

import numpy as np
import concourse.bass as bass
import concourse.bacc as bacc
import concourse.mybir as mybir
from concourse import ap_utils
from concourse.bass import exact_div
from concourse.tile import TileContext
from concourse.masks import make_identity

import jax
from jax.sharding import Mesh, PartitionSpec
from jax.experimental.shard_map import shard_map
from concourse.bass2jax import (_bass_exec_p, install_neuronx_cc_hook,
                                partition_id_tensor)

FP = mybir.dt.float32
F16 = mybir.dt.float16
AF = mybir.ActivationFunctionType
OP = mybir.AluOpType
AX = mybir.AxisListType

N_NODES = 100000
N_CORES = 8
D_NBR = 32
K1, F1 = 8, 8
K2, F2 = 8, 16
NEG_SLOPE = 0.01

S_CORE = N_NODES // N_CORES          # 12500
N_TILES = (S_CORE + 127) // 128      # 98
SP = N_TILES * 128                   # 12544 (padded dests per core)
NP = N_CORES * SP                    # 100352 (table rows: padded ids)
N_CHUNKS = NP // 128                 # 784
QN = NP // 4                         # 25088 rows per index quarter
AG_TILES = (25, 25, 25, 23)          # A1-tile split for pipelined AllGather
AG_OFF = (0, 25, 50, 75)
NI = D_NBR + 1                       # 33 layer-1 slots (slot 0 = self)
H1, H2 = K1 * F1, K2 * F2            # 64, 128
R1 = H1 + 16                         # 80  layer-1 row [h1|s1|s2]
R2S = 256                            # table2 row stride (512 B)
GE2 = H2 + K2                        # 136 gathered elems [h2|s2]
Q1 = H1 + K1                         # 72 layer-1 tree width
Q2 = H2 + K2                         # 136 layer-2 tree width
TG = 8                               # chunks per T2 DMA group


def _gather_raw(nc, out_ap, in_ap, idxs_ap, num_idxs, elem_size, elem_step,
                queue_num=0):
    """nc.gpsimd.dma_gather minus the %256 elem-size assert (the Q7 ucode
    packets arbitrary elem sizes; only the row stride must be %256B)."""
    g = nc.gpsimd
    assert idxs_ap.dtype == mybir.dt.int16
    assert in_ap.dtype == out_ap.dtype
    assert ap_utils.ap_is_contiguous(out_ap.ap[1:])
    assert ap_utils.ap_is_contiguous(idxs_ap.ap[1:])
    assert in_ap.ap[-1][1] == out_ap.ap[-1][1] == elem_size
    assert out_ap.ap[0][1] * out_ap.ap[1][1] == num_idxs
    assert in_ap.ap[0][0] == elem_step
    stride_bytes = elem_step * mybir.dt.size(in_ap.dtype)
    stride_bytes_256 = exact_div(stride_bytes, 256)
    _in_ap = g.lower_ap_dma(in_ap, for_custom_bir_dma=True)
    _idxs_ap = g.lower_ap(idxs_ap)
    _out_ap = g.lower_ap(out_ap)
    return g.add_instruction(
        mybir.InstDMAGatherAnt(
            name=g.bass.get_next_instruction_name(),
            ins=[*_in_ap, _idxs_ap, g.lower_val_access(g.to_reg(num_idxs))],
            outs=[_out_ap],
            transpose=False,
            num_idxs=num_idxs,
            elem_size=elem_size,
            stride_bytes_256=stride_bytes_256,
            gen_mode=0,
            single_packet=False,
            queue_num=queue_num,
            sbuf_tokens_per_rank=0,
            sbuf_free_dim_per_rank=0,
            sbuf_free_dim_pad_per_rank=0,
            sbuf_byte_offset=0,
        ))


def _build_gat(sq, s_alloc):
    """sq: [N_TILES][4] padded slots per quarter (unified over cores);
    s_alloc: [N_TILES] staging slots (sum(sq[t]) rounded up to %8)."""
    import os
    phases = os.environ.get('KV3_PHASES', 'all')
    _build_gat.rr = 0
    s_cap = max(s_alloc)
    stot = [sum(sq[t]) for t in range(N_TILES)]
    ioff = np.concatenate([[0], np.cumsum([8 * s for s in stot])])
    moff = np.concatenate([[0], np.cumsum(s_alloc)])

    nc = bacc.Bacc("TRN2", target_bir_lowering=False, debug=False,
                   num_devices=N_CORES, dynamic_dma_scratch_size=32768,
                   num_swdge_queues=4)
    xeT = nc.dram_tensor("xeT", [128, N_TILES * NI * 128], F16,
                         kind="ExternalInput").ap()
    rhs1 = nc.dram_tensor("rhs1", [128, R1], F16, kind="ExternalInput").ap()
    rhs2 = nc.dram_tensor("rhs2", [H1, 144], F16, kind="ExternalInput").ap()
    idx16 = nc.dram_tensor("idx16", [128, int(ioff[-1])], mybir.dt.int16,
                           kind="ExternalInput").ap()
    maskd = nc.dram_tensor("maskd", [128, int(moff[-1])], F16,
                           kind="ExternalInput").ap()
    selfid = nc.dram_tensor("selfid", [128, N_TILES], mybir.dt.int32,
                            kind="ExternalInput").ap()
    out = nc.dram_tensor("out", [S_CORE, F2], FP, kind="ExternalOutput").ap()

    table2 = nc.dram_tensor("table2", [NP, R2S], F16).ap()
    x2T_shard = nc.dram_tensor("x2T_shard", [H1, SP], F16).ap()
    x2T_all = [nc.dram_tensor(f"x2T_all{p}", [N_CORES * H1, 128 * AG_TILES[p]],
                              F16, addr_space="Shared").ap()
               for p in range(4)]
    x2T_bounce = [nc.dram_tensor(f"x2T_bounce{p}", [H1, 128 * AG_TILES[p]],
                                 F16).ap()
                  for p in range(4)]

    with TileContext(nc) as tc:
        with tc.tile_pool(name="const", bufs=1) as cpool, \
             tc.tile_pool(name="a1", bufs=2) as a1p, \
             tc.tile_pool(name="tb", bufs=2) as tbp, \
             tc.tile_pool(name="a2", bufs=2) as a2p, \
             tc.tile_pool(name="ps_mm", bufs=4, space="PSUM") as pp, \
             tc.tile_pool(name="ps_tr", bufs=2, space="PSUM") as pp_tr, \
             tc.tile_pool(name="ps_t2", bufs=2, space="PSUM") as pp_t2:

            rt1 = cpool.tile([128, R1], F16)
            nc.sync.dma_start(out=rt1[:], in_=rhs1[:, :])
            rt2 = cpool.tile([H1, 144], F16)
            nc.sync.dma_start(out=rt2[:], in_=rhs2[:, :])
            ident = cpool.tile([128, 128], FP)
            make_identity(nc, ident[:])
            mall = cpool.tile([128, int(moff[-1])], F16)
            nc.sync.dma_start(out=mall[:], in_=maskd[:, :])
            sall = cpool.tile([128, N_TILES], mybir.dt.int32)
            nc.sync.dma_start(out=sall[:], in_=selfid[:, :])

            # ---- phase A1: layer-1 attention, host-expanded edges ----
            for t in range(N_TILES):
                r0 = t * 128
                M = min(128, S_CORE - r0)
                lt = a1p.tile([128, NI * 128], F16, name=f"a1l{t}", tag="a1l")
                nc.sync.dma_start(
                    out=lt[:], in_=xeT[:, t * NI * 128:(t + 1) * NI * 128])
                hg = a1p.tile([128, NI * R1], F16, name=f"a1g{t}", tag="a1g")
                hgv = hg[:].rearrange("p (n r) -> p n r", r=R1)
                for gi, j0 in enumerate(range(0, NI, 6)):
                    n6 = min(6, NI - j0)
                    ps = pp.tile([128, n6 * R1], FP, name=f"a1p{t}_{j0}",
                                 tag="a1p", space="PSUM")
                    for j in range(j0, j0 + n6):
                        nc.tensor.matmul(out=ps[:, (j - j0) * R1:
                                                 (j - j0 + 1) * R1],
                                         lhsT=lt[:, j * 128:(j + 1) * 128],
                                         rhs=rt1[:], start=True, stop=True)
                    if gi % 2 == 0:
                        nc.vector.tensor_copy(
                            out=hg[:, j0 * R1:(j0 + n6) * R1], in_=ps[:])
                    else:
                        nc.scalar.activation(
                            out=hg[:, j0 * R1:(j0 + n6) * R1], in_=ps[:],
                            func=AF.Copy)
                u = a1p.tile([128, D_NBR * K1], F16, name=f"a1u{t}", tag="a1u")
                nc.vector.tensor_tensor(
                    out=u[:].rearrange("p (d k) -> p d k", k=K1),
                    in0=hgv[:, 1:, H1 + 8:H1 + 16],
                    in1=hgv[:, 0:1, H1:H1 + 8].to_broadcast([128, D_NBR, K1]),
                    op=OP.add)
                nc.scalar.activation(out=u[:], in_=u[:], func=AF.Prelu,
                                     alpha=NEG_SLOPE)
                nc.scalar.activation(out=u[:], in_=u[:], func=AF.Exp)
                v = a1p.tile([128, D_NBR * Q1], F16, name=f"a1v{t}", tag="a1v")
                vv = v[:].rearrange("p (d q) -> p d q", q=Q1)
                u3d = u[:].rearrange("p (d k) -> p d k", k=K1)
                nc.vector.tensor_tensor(
                    out=vv[:, :, 0:H1].rearrange("p d (k f) -> p d k f", f=F1),
                    in0=hgv[:, 1:, 0:H1].rearrange("p d (k f) -> p d k f",
                                                   f=F1),
                    in1=u3d.unsqueeze(3).to_broadcast([128, D_NBR, K1, F1]),
                    op=OP.mult)
                nc.vector.tensor_copy(out=vv[:, :, H1:Q1], in_=u3d)
                w1 = a1p.tile([128, 16 * Q1], F16, name=f"a1w1{t}", tag="a1w1")
                nc.vector.tensor_tensor(out=w1[:], in0=v[:, :16 * Q1],
                                        in1=v[:, 16 * Q1:32 * Q1], op=OP.add)
                w2 = a1p.tile([128, 8 * Q1], F16, name=f"a1w2{t}", tag="a1w2")
                nc.vector.tensor_tensor(out=w2[:], in0=w1[:, :8 * Q1],
                                        in1=w1[:, 8 * Q1:16 * Q1], op=OP.add)
                w3 = a1p.tile([128, 4 * Q1], F16, name=f"a1w3{t}", tag="a1w3")
                nc.vector.tensor_tensor(out=w3[:], in0=w2[:, :4 * Q1],
                                        in1=w2[:, 4 * Q1:8 * Q1], op=OP.add)
                w4 = a1p.tile([128, 2 * Q1], F16, name=f"a1w4{t}", tag="a1w4")
                nc.vector.tensor_tensor(out=w4[:], in0=w3[:, :2 * Q1],
                                        in1=w3[:, 2 * Q1:4 * Q1], op=OP.add)
                st = a1p.tile([128, Q1], FP, name=f"a1s{t}", tag="a1s")
                nc.vector.tensor_tensor(out=st[:], in0=w4[:, :Q1],
                                        in1=w4[:, Q1:2 * Q1], op=OP.add)
                rz = a1p.tile([128, K1], FP, name=f"a1rz{t}", tag="a1rz")
                nc.vector.reciprocal_approx_fast(out=rz[:], in_=st[:, H1:Q1])
                o = a1p.tile([128, H1], FP, name=f"a1o{t}", tag="a1o")
                nc.vector.tensor_tensor(
                    out=o[:].rearrange("p (k f) -> p k f", f=F1),
                    in0=st[:, 0:H1].rearrange("p (k f) -> p k f", f=F1),
                    in1=rz[:].unsqueeze(2).to_broadcast([128, K1, F1]),
                    op=OP.mult)
                mn = a1p.tile([128, H1], FP, name=f"a1m{t}", tag="a1m")
                nc.vector.tensor_scalar_min(out=mn[:], in0=o[:], scalar1=0.0)
                nc.scalar.activation(out=mn[:], in_=mn[:], func=AF.Exp)
                x2 = a1p.tile([128, H1], FP, name=f"a1x{t}", tag="a1x")
                nc.vector.scalar_tensor_tensor(
                    out=x2[:], in0=mn[:], scalar=-1.0, in1=o[:],
                    op0=OP.add, op1=OP.max)
                pt = pp_tr.tile([H1, 128], FP, name=f"a1pt{t}",
                                tag="a1pt", space="PSUM")
                nc.tensor.transpose(out=pt[:], in_=x2[:], identity=ident[:])
                xt = a1p.tile([H1, 128], F16, name=f"a1xt{t}", tag="a1xt")
                nc.vector.tensor_copy(out=xt[:], in_=pt[:])
                nc.sync.dma_start(out=x2T_shard[:, r0:r0 + 128], in_=xt[:])

                # pipelined AllGather: ship finished column parts during A1
                if phases != 'a1' and t + 1 in (25, 50, 75, 98):
                    p = (25, 50, 75, 98).index(t + 1)
                    c0, w = AG_OFF[p] * 128, AG_TILES[p] * 128
                    nc.sync.dma_start(out=x2T_bounce[p][:, :],
                                      in_=x2T_shard[:, c0:c0 + w])
                    nc.gpsimd.collective_compute(
                        "AllGather", OP.bypass,
                        replica_groups=[list(range(N_CORES))],
                        ins=[x2T_bounce[p].opt()], outs=[x2T_all[p].opt()])

            # ---- phase T2: table2 rows [s1|h2|s2] @ 512B stride ----
            if phases == 'a1':
                zz = a1p.tile([128, F2], FP, name="zz", tag="zz")
                nc.vector.memset(zz[:], 0.0)
                nc.sync.dma_start(out=out[0:128, :], in_=zz[:])

            def t2_part(c):
                """global chunk -> (core r, part p, col within part)."""
                r, lc = divmod(c, N_TILES)
                p = min(lc // 25, 3)
                return r, p, (lc - AG_OFF[p]) * 128

            def t2_load(lt2, g0c, nck):
                """Load chunks [g0c, g0c+nck) merging runs within one
                (core, part) block."""
                s = 0
                while s < nck:
                    r, p, col = t2_part(g0c + s)
                    e = s + 1
                    while e < nck:
                        r2, p2, col2 = t2_part(g0c + e)
                        if (r2, p2) != (r, p) or col2 != col + (e - s) * 128:
                            break
                        e += 1
                    nc.sync.dma_start(
                        out=lt2[:, s * 128:e * 128],
                        in_=x2T_all[p][r * H1:(r + 1) * H1,
                                       col:col + (e - s) * 128])
                    s = e

            n_grp2 = (N_CHUNKS // TG) if phases != 'a1' else 0
            for g in range(n_grp2):
                g0 = g * TG * 128
                lt2 = tbp.tile([H1, TG * 128], F16, name=f"t2l{g}", tag="t2l")
                t2_load(lt2, g * TG, TG)
                rows2 = tbp.tile([128, TG * 144], F16, name=f"t2r{g}",
                                 tag="t2r")
                for cc in range(0, TG, 2):
                    ps2 = pp_t2.tile([128, 288], FP, name=f"t2p{g}_{cc}",
                                     tag="t2p", space="PSUM")
                    for k in range(2):
                        nc.tensor.matmul(
                            out=ps2[:, k * 144:(k + 1) * 144],
                            lhsT=lt2[:, (cc + k) * 128:(cc + k + 1) * 128],
                            rhs=rt2[:], start=True, stop=True)
                    if cc % 4 == 0:
                        nc.vector.tensor_copy(
                            out=rows2[:, cc * 144:(cc + 2) * 144], in_=ps2[:])
                    else:
                        nc.scalar.activation(
                            out=rows2[:, cc * 144:(cc + 2) * 144], in_=ps2[:],
                            func=AF.Copy)
                nc.sync.dma_start(
                    out=table2[g0:g0 + TG * 128, 0:144]
                        .rearrange("(c p) r -> p c r", p=128),
                    in_=rows2[:].rearrange("p (c r) -> p c r", r=144))

            # ---- phase A2: quarter-split gathers + attention ----
            if phases == 't2':
                zz = a1p.tile([128, F2], FP, name="zz2", tag="zz")
                nc.vector.memset(zz[:], 0.0)
                nc.sync.dma_start(out=out[0:128, :], in_=zz[:])
            for t in range(N_TILES if phases == 'all' else 0):
                r0 = t * 128
                M = min(128, S_CORE - r0)
                SQt, SA = sq[t], s_alloc[t]
                ST = sum(SQt)
                it2 = a2p.tile([128, 8 * s_cap], mybir.dt.int16,
                               name=f"a2i{t}", tag="a2i")
                nc.sync.dma_start(out=it2[:, :8 * ST],
                                  in_=idx16[:, int(ioff[t]):int(ioff[t]) +
                                            8 * ST])
                hg2 = a2p.tile([128, s_cap * GE2], F16, name=f"a2g{t}",
                               tag="a2g")
                if t < 2:
                    nc.vector.memset(hg2[:], 0.0)
                hg2v = hg2[:].rearrange("p (s e) -> p s e", e=GE2)
                off = 0
                xoff = 0
                for q in range(4):
                    for c0 in range(0, SQt[q], 15):
                        sub = min(15, SQt[q] - c0)
                        _gather_raw(
                            nc,
                            out_ap=hg2v[:, off + c0:off + c0 + sub, :],
                            in_ap=table2[q * QN:(q + 1) * QN, 8:8 + GE2],
                            idxs_ap=it2[:, xoff + 8 * c0:xoff + 8 * (c0 + sub)],
                            num_idxs=128 * sub, elem_size=GE2, elem_step=R2S,
                            queue_num=_build_gat.rr % 4)
                        _build_gat.rr += 1
                    off += SQt[q]
                    xoff += 8 * SQt[q]
                s1d = a2p.tile([128, K2], F16, name=f"a2sd{t}", tag="a2sd")
                nc.gpsimd.indirect_dma_start(
                    out=s1d[:], out_offset=None, in_=table2[:],
                    in_offset=bass.IndirectOffsetOnAxis(
                        ap=sall[:, t:t + 1], axis=0))
                u2 = a2p.tile([128, s_cap * K2], F16, name=f"a2u{t}",
                              tag="a2u")
                u2v = u2[:].rearrange("p (s k) -> p s k", k=K2)
                nc.vector.tensor_tensor(
                    out=u2v[:, :SA, :], in0=hg2v[:, :SA, H2:H2 + 8],
                    in1=s1d[:].unsqueeze(1).to_broadcast([128, SA, K2]),
                    op=OP.add)
                nc.scalar.activation(out=u2[:, :SA * K2], in_=u2[:, :SA * K2],
                                     func=AF.Prelu, alpha=NEG_SLOPE)
                nc.scalar.activation(out=u2[:, :SA * K2], in_=u2[:, :SA * K2],
                                     func=AF.Exp)
                nc.vector.tensor_tensor(
                    out=u2v[:, :SA, :], in0=u2v[:, :SA, :],
                    in1=mall[:, int(moff[t]):int(moff[t]) + SA]
                        .unsqueeze(2).to_broadcast([128, SA, K2]),
                    op=OP.mult)
                v2 = a2p.tile([128, s_cap * Q2], F16, name=f"a2v{t}",
                              tag="a2v")
                vv2 = v2[:].rearrange("p (s q) -> p s q", q=Q2)
                nc.vector.tensor_tensor(
                    out=vv2[:, :SA, 0:H2].rearrange("p s (k f) -> p s k f",
                                                    f=F2),
                    in0=hg2v[:, :SA, 0:H2].rearrange("p s (k f) -> p s k f",
                                                     f=F2),
                    in1=u2v[:, :SA, :].unsqueeze(3)
                        .to_broadcast([128, SA, K2, F2]),
                    op=OP.mult)
                nc.vector.tensor_copy(out=vv2[:, :SA, H2:Q2],
                                      in_=u2v[:, :SA, :])
                # tree: SA (=8m) -> SA/2 -> SA/4 -> SA/8 = m, then reduce m
                w1 = a2p.tile([128, (s_cap // 2) * Q2], F16, name=f"a2w1{t}",
                              tag="a2w1")
                h = SA // 2
                nc.vector.tensor_tensor(out=w1[:, :h * Q2], in0=v2[:, :h * Q2],
                                        in1=v2[:, h * Q2:2 * h * Q2],
                                        op=OP.add)
                h2_ = SA // 4
                nc.vector.tensor_tensor(out=v2[:, :h2_ * Q2],
                                        in0=w1[:, :h2_ * Q2],
                                        in1=w1[:, h2_ * Q2:2 * h2_ * Q2],
                                        op=OP.add)
                m = SA // 8
                nc.vector.tensor_tensor(out=w1[:, :m * Q2],
                                        in0=v2[:, :m * Q2],
                                        in1=v2[:, m * Q2:2 * m * Q2],
                                        op=OP.add)
                st2 = a2p.tile([128, Q2], FP, name=f"a2s{t}", tag="a2s")
                if m == 1:
                    nc.vector.tensor_copy(out=st2[:], in_=w1[:, :Q2])
                else:
                    nc.vector.tensor_reduce(
                        out=st2[:],
                        in_=w1[:, :m * Q2]
                            .rearrange("p (s q) -> p s q", q=Q2)
                            .transpose([0, 2, 1]),
                        axis=AX.X, op=OP.add)
                rz2 = a2p.tile([128, K2], FP, name=f"a2rz{t}", tag="a2rz")
                nc.vector.reciprocal_approx_fast(out=rz2[:], in_=st2[:, H2:Q2])
                o2 = a2p.tile([128, H2], FP, name=f"a2o{t}", tag="a2o")
                nc.vector.tensor_tensor(
                    out=o2[:].rearrange("p (k f) -> p k f", f=F2),
                    in0=st2[:, 0:H2].rearrange("p (k f) -> p k f", f=F2),
                    in1=rz2[:].unsqueeze(2).to_broadcast([128, K2, F2]),
                    op=OP.mult)
                mo = a2p.tile([128, F2], FP, name=f"a2mo{t}", tag="a2mo")
                nc.vector.tensor_reduce(
                    out=mo[:],
                    in_=o2[:].rearrange("p (k f) -> p k f", f=F2)
                        .transpose([0, 2, 1]),
                    axis=AX.X, op=OP.add)
                u3 = a2p.tile([128, F2], FP, name=f"a2u3{t}", tag="a2u3")
                z3 = a2p.tile([128, 1], FP, name=f"a2z3{t}", tag="a2z3")
                nc.scalar.activation(out=u3[:], in_=mo[:], func=AF.Exp,
                                     scale=1.0 / K2, accum_out=z3[:])
                rz3 = a2p.tile([128, 1], FP, name=f"a2rz3{t}", tag="a2rz3")
                nc.vector.reciprocal_approx_fast(out=rz3[:], in_=z3[:])
                ot = a2p.tile([128, F2], FP, name=f"a2ot{t}", tag="a2ot")
                nc.vector.tensor_tensor(
                    out=ot[:], in0=u3[:],
                    in1=rz3[:].to_broadcast([128, F2]), op=OP.mult)
                nc.sync.dma_start(out=out[r0:r0 + M, :], in_=ot[:M, :])

    nc.finalize()
    return nc


class _SpmdRunner:
    """jit-once SPMD executor over the 8 axon NeuronCores."""

    def __init__(self, nc, n_cores):
        install_neuronx_cc_hook()
        self.nc, self.n_cores = nc, n_cores
        partition_name = (nc.partition_id_tensor.name
                          if nc.partition_id_tensor else None)
        in_names, out_names, out_avals, zero_outs = [], [], [], []
        for alloc in nc.m.functions[0].allocations:
            if not isinstance(alloc, mybir.MemoryLocationSet):
                continue
            name = alloc.memorylocations[0].name
            if alloc.kind == "ExternalInput":
                if name != partition_name:
                    in_names.append(name)
            elif alloc.kind == "ExternalOutput":
                out_names.append(name)
                shape = tuple(alloc.tensor_shape)
                dtype = mybir.dt.np(alloc.dtype)
                out_avals.append(jax.core.ShapedArray(shape, dtype))
                zero_outs.append(np.zeros(shape, dtype))
        self.in_names, self.out_names = in_names, out_names
        self.out_avals, self.zero_outs = out_avals, zero_outs
        all_in_names = in_names + out_names
        if partition_name is not None:
            all_in_names.append(partition_name)

        def _body(*args):
            operands = list(args)
            if partition_name is not None:
                operands.append(partition_id_tensor())
            return tuple(_bass_exec_p.bind(
                *operands, out_avals=tuple(out_avals),
                in_names=tuple(all_in_names), out_names=tuple(out_names),
                lowering_input_output_aliases=(),
                sim_require_finite=True, sim_require_nnan=True, nc=nc))

        devices = jax.devices()[:n_cores]
        self.mesh = Mesh(np.asarray(devices), ("core",))
        n_params, n_outs = len(in_names), len(out_avals)
        in_specs = (PartitionSpec("core"),) * (n_params + n_outs)
        out_specs = (PartitionSpec("core"),) * n_outs
        self.fn = jax.jit(
            shard_map(_body, mesh=self.mesh, in_specs=in_specs,
                      out_specs=out_specs, check_rep=False),
            keep_unused=True)
        self.sharding = jax.sharding.NamedSharding(self.mesh,
                                                   PartitionSpec("core"))

    def run(self, in_maps):
        per_core = [[np.asarray(m[n]) for n in self.in_names] for m in in_maps]
        concat = [np.concatenate([per_core[c][i] for c in range(self.n_cores)],
                                 axis=0) for i in range(len(self.in_names))]
        zeros = [np.zeros((self.n_cores * z.shape[0], *z.shape[1:]), z.dtype)
                 for z in self.zero_outs]
        dev = [jax.device_put(a, self.sharding) for a in concat + zeros]
        outs = self.fn(*dev)
        jax.block_until_ready(outs)
        res = []
        for c in range(self.n_cores):
            res.append({name: np.asarray(outs[i]).reshape(
                self.n_cores, *self.out_avals[i].shape)[c]
                for i, name in enumerate(self.out_names)})
        return res


def _wrap16(lst):
    """[n] -> [128, n//16]: wrapped in 16 partitions, replicated x8 groups."""
    n = len(lst)
    w = np.asarray(lst, np.int16).reshape(n // 16, 16).T  # [16, n//16]
    return np.tile(w, (8, 1))


def _plan(neighbors):
    """Data-dependent plan: per-core permutation + unified per-tile quarter
    widths + per-core idx16/mask/selfid arrays."""
    nbr0 = np.asarray(neighbors, np.int64).reshape(N_CORES, S_CORE, D_NBR)
    # padded table ids: node (core c, local i) -> c*SP + i
    nbr = (nbr0 // S_CORE) * SP + nbr0 % S_CORE
    qq = (nbr // QN).astype(np.int32)                     # [8, S, 32]
    counts = np.zeros((N_CORES, S_CORE, 4), np.int32)
    for q in range(4):
        counts[:, :, q] = (qq == q).sum(axis=2)
    perms = []
    percore_cnt = []
    for r in range(N_CORES):
        key = counts[r].max(axis=1)
        perm = np.argsort(-key, kind='stable').astype(np.int32)
        perm_pad = np.concatenate(
            [perm, np.full(SP - S_CORE, perm[-1], np.int32)])
        perms.append(perm_pad)
        percore_cnt.append(counts[r][perm_pad])           # [SP, 4]
    # unified per-tile quarter widths
    sq = []
    for t in range(N_TILES):
        mx = np.zeros(4, np.int64)
        for r in range(N_CORES):
            mx = np.maximum(mx, percore_cnt[r][t * 128:(t + 1) * 128]
                            .max(axis=0))
        sq.append([int(x) for x in mx])
    s_alloc = [max(8, -(-sum(s) // 8) * 8) for s in sq]

    idx16s, masks, selfids = [], [], []
    for r in range(N_CORES):
        perm_pad = perms[r]
        srt = np.sort(nbr[r], axis=1)                     # [S, 32] sorted src
        srt_pad = srt[perm_pad]                           # [SP, 32]
        cnt_pad = percore_cnt[r]                          # [SP, 4]
        start = np.concatenate([np.zeros((SP, 1), np.int64),
                                np.cumsum(cnt_pad, axis=1)[:, :3]], axis=1)
        iblocks, mblocks = [], []
        for t in range(N_TILES):
            rows = slice(t * 128, (t + 1) * 128)
            mtile = np.zeros((128, s_alloc[t]), np.float16)
            off = 0
            for q in range(4):
                SQ = sq[t][q]
                if SQ == 0:
                    continue
                k = np.arange(SQ)[None, :]
                gidx = start[rows, q:q + 1] + k
                valid = k < cnt_pad[rows, q:q + 1]
                vals = np.take_along_axis(
                    srt_pad[rows], np.minimum(gidx, D_NBR - 1), axis=1)
                vals = np.where(valid, vals - q * QN, 0).astype(np.int16)
                iblocks.append(_wrap16(vals.T.ravel()))
                mtile[:, off:off + SQ] = valid.astype(np.float16)
                off += SQ
            mblocks.append(mtile)
        idx16s.append(np.concatenate(iblocks, axis=1))
        masks.append(np.concatenate(mblocks, axis=1))
        gid = (r * SP + perm_pad).astype(np.int32)        # [SP] padded ids
        selfids.append(np.ascontiguousarray(
            gid.reshape(N_TILES, 128).T))                 # [128, N_TILES]
    return sq, s_alloc, perms, idx16s, masks, selfids


def _host_prep(node_features, neighbors, W1, a1_1, a2_1, W2, a1_2, a2_2,
               plan):
    sq, s_alloc, perms, idx16s, masks, selfids = plan

    def blk(a, k, f):
        A = np.zeros((k * f, k), np.float32)
        for kk in range(k):
            A[kk * f:(kk + 1) * f, kk] = a[kk]
        return A

    rhs1 = np.concatenate(
        [W1, W1 @ blk(a1_1, K1, F1), W1 @ blk(a2_1, K1, F1)],
        axis=1).astype(np.float16)                        # [128, 80] h|s1|s2
    rhs2 = np.concatenate(
        [W2 @ blk(a1_2, K2, F2), W2, W2 @ blk(a2_2, K2, F2)],
        axis=1).astype(np.float16)                        # [64, 144] s1|h2|s2
    xT = node_features.T.astype(np.float16)               # [128, N]
    nbr = np.asarray(neighbors, np.int64)

    in_maps = []
    for r in range(N_CORES):
        ids = np.arange(r * S_CORE, (r + 1) * S_CORE, dtype=np.int64)
        nb33 = np.concatenate([ids[:, None], nbr[r * S_CORE:(r + 1) * S_CORE]],
                              axis=1)                     # [S, 33]
        nb33 = np.concatenate(
            [nb33, np.zeros((SP - S_CORE, NI), np.int64)], axis=0)
        # xeT[:, t*NI*128 + j*128 + p] = xT[:, nb33[t*128+p, j]]
        flat = nb33.reshape(N_TILES, 128, NI).transpose(0, 2, 1).ravel()
        xeT = np.ascontiguousarray(xT[:, flat])
        in_maps.append({'xeT': xeT, 'rhs1': rhs1, 'rhs2': rhs2,
                        'idx16': idx16s[r], 'maskd': masks[r],
                        'selfid': selfids[r]})
    return in_maps


_RUNNER = None
_PLAN = None
_PLAN_KEY = None


def _get_runner(neighbors):
    global _RUNNER, _PLAN, _PLAN_KEY
    key = hash(np.asarray(neighbors, np.int64).tobytes())
    if _RUNNER is None or key != _PLAN_KEY:
        _PLAN = _plan(neighbors)
        _PLAN_KEY = key
        nc = _build_gat(_PLAN[0], _PLAN[1])
        _RUNNER = _SpmdRunner(nc, N_CORES)
    return _RUNNER, _PLAN


def kernel(node_features, neighbors, W1, a1_1, a2_1, W2, a1_2, a2_2):
    node_features = np.asarray(node_features, dtype=np.float32)
    neighbors = np.asarray(neighbors)
    runner, plan = _get_runner(neighbors)
    in_maps = _host_prep(node_features, neighbors,
                         np.asarray(W1, np.float32),
                         np.asarray(a1_1, np.float32),
                         np.asarray(a2_1, np.float32),
                         np.asarray(W2, np.float32),
                         np.asarray(a1_2, np.float32),
                         np.asarray(a2_2, np.float32), plan)
    res = runner.run(in_maps)
    out = np.empty((N_NODES, F2), np.float32)
    for r in range(N_CORES):
        perm = plan[2][r][:S_CORE]
        out[r * S_CORE + perm] = res[r]['out']
    return out



# revision 53
# speedup vs baseline: 15.2875x; 15.2875x over previous
"""2-layer GAT (100000 nodes, 32 neighbors) on 8 trn2 NeuronCores — v3.1.

v3.1 changes over v3 (cost-model timeline: 3245us -> 2483us):
  - Lrelu -> Prelu: AF.Lrelu lives only in the derivative_gelu act-table set,
    so every Lrelu<->Exp switch reloaded the activation table (391 reloads,
    ~500us of Act engine time). AF.Prelu (same math, alpha operand) shares
    the exp_and_others set with Exp AND Copy -> zero reloads.
  - Layer-1 matmuls pack 6 slots into one [128,480] PSUM tile with a single
    PSUM->SBUF copy per group (33 copies -> 6, alternating DVE/Act).
  - table2 rows are padded ids (core*12544+i, 100352 rows) so the x2T_all ->
    table2 chunk mapping never straddles cores; quarter size 25088.
  - The x2->table2 AllGather is split into 4 column parts (25/25/25/23
    A1-tiles) issued inside the A1 loop, hiding ~2/3 of the collective
    under layer-1 compute; T2 loads run-merge across part buffers.
  - T2 packs 2 chunks per [128,288] PSUM tile, copies alternate DVE/Act.

Profiling: no NTFF here; use work/profile_tl.py (TimelineSim + cost model,
single core). Engine busy% there: DVE 76% (bottleneck), DMA 56%, Pool 30%.

Negative results verified this session (do not retry without new evidence):
  - indirect_dma_start with [128,N>1] int32 offset APs returns garbage on
    real ucode (work/probe_indirect.py: not even a permutation of the
    requested rows) despite bass_interp modeling it fine. Only [128,1]
    offsets and the int16 dma_gather path work on HW.
  - nc.gpsimd.tensor_copy reading PSUM (and/or Act Copy with a strided 3D
    out AP) fails neuronx-cc with an opaque CallFunctionObjArgs error.
  - dynamic_dma_scratch_size=65536 overflows SBUF (a2 pool ~114KB/part).
  - Layer-1 restructure (X1 transform -> single AllGather table1 -> int16
    quarter gathers, shared idx16 via inverse-perm values; implemented and
    built OK) REGRESSES the model 2483->3340us: the AG1 barrier kills the
    v3.1 overlap where A2's ~640us gather dispatch + early T2 run UNDER
    the A1 matmul phase (A2 gathers start at ~940us, A1 ends ~1160us).
    Removing A1's 3234 matmuls is worth less than that overlap. To win,
    AG1 must be pipelined in 4 parts, which forces part-major table1 rows
    and a SECOND idx16/mask set for layer 1 (quarter!=core//2 then);
    est. net -150..-250us model, needs ~45 min + 2 HW cycles.
The 33-matmul A1 looks wasteful but is load-bearing for overlap.
"""
"""Original v3 notes:

Architecture (vs 10.85 ms baseline, which was SWDGE-descriptor-bound:
33 indirect DMAs x ~1 us Pool fixed cost per 128-node tile, twice):

  Layer 1 — NO gathers: the neighbor lists are static inputs, so the host
  pre-expands transposed node features per edge slot (xeT [128, 98*33*128]
  fp16). Each tile streams its slice and recomputes [h1|s1|s2] per slot with
  33 PE matmuls against the fused rhs1 = [W1 | W1@A1 | W1@A2]. PE is
  otherwise idle; Pool is not involved at all.

  Layer 2 — table2 [100096 rows x 512B stride] fp16 rows [s1(8)|h2(128)|s2(8)]
  built from the AllGathered x2^T, then gathered per destination tile with
  dma_gather (batched SWDGE gather, ~0.34ns/descriptor) in 4 quarter-split
  passes (int16 indices are quarter-local, < 25024). Slots are sorted by
  source within each partition (softmax is order-invariant), padded
  per-quarter to the tile max with dummy index 0, and masked after exp.
  Destinations are bin-packed into tiles (sorted by worst quarter count,
  unified across the 8 cores) to minimize padding; the host un-permutes the
  output rows at the end.

  Attention math runs in gather-natural d-major fp16 layout with contiguous
  DVE ops and a pairwise halving tree (softmax denominator rides in 8 extra
  tree columns).

The Bass program is built on the first kernel() call because tile shapes
depend on the neighbor data (per-tile padded quarter widths).
"""
import sys

if '/opt/trn_rl_repo' not in sys.path:
    sys.path.insert(0, '/opt/trn_rl_repo')

import numpy as np
import concourse.bass as bass
import concourse.bacc as bacc
import concourse.mybir as mybir
from concourse import ap_utils
from concourse.bass import exact_div
from concourse.tile import TileContext
from concourse.masks import make_identity

import jax
from jax.sharding import Mesh, PartitionSpec
from jax.experimental.shard_map import shard_map
from concourse.bass2jax import (_bass_exec_p, install_neuronx_cc_hook,
                                partition_id_tensor)

FP = mybir.dt.float32
F16 = mybir.dt.float16
AF = mybir.ActivationFunctionType
OP = mybir.AluOpType
AX = mybir.AxisListType

N_NODES = 100000
N_CORES = 8
D_NBR = 32
K1, F1 = 8, 8
K2, F2 = 8, 16
NEG_SLOPE = 0.01

S_CORE = N_NODES // N_CORES          # 12500
N_TILES = (S_CORE + 127) // 128      # 98
SP = N_TILES * 128                   # 12544 (padded dests per core)
NP = N_CORES * SP                    # 100352 (table rows: padded ids)
N_CHUNKS = NP // 128                 # 784
QN = NP // 4                         # 25088 rows per index quarter
AG_TILES = (25, 25, 25, 23)          # A1-tile split for pipelined AllGather
AG_OFF = (0, 25, 50, 75)
NI = D_NBR + 1                       # 33 layer-1 slots (slot 0 = self)
H1, H2 = K1 * F1, K2 * F2            # 64, 128
R1 = H1 + 16                         # 80  layer-1 row [h1|s1|s2]
R2S = 256                            # table2 row stride (512 B)
GE2 = H2 + K2                        # 136 gathered elems [h2|s2]
Q1 = H1 + K1                         # 72 layer-1 tree width
Q2 = H2 + K2                         # 136 layer-2 tree width
TG = 8                               # chunks per T2 DMA group


def _gather_raw(nc, out_ap, in_ap, idxs_ap, num_idxs, elem_size, elem_step,
                queue_num=0):
    """nc.gpsimd.dma_gather minus the %256 elem-size assert (the Q7 ucode
    packets arbitrary elem sizes; only the row stride must be %256B)."""
    g = nc.gpsimd
    assert idxs_ap.dtype == mybir.dt.int16
    assert in_ap.dtype == out_ap.dtype
    assert ap_utils.ap_is_contiguous(out_ap.ap[1:])
    assert ap_utils.ap_is_contiguous(idxs_ap.ap[1:])
    assert in_ap.ap[-1][1] == out_ap.ap[-1][1] == elem_size
    assert out_ap.ap[0][1] * out_ap.ap[1][1] == num_idxs
    assert in_ap.ap[0][0] == elem_step
    stride_bytes = elem_step * mybir.dt.size(in_ap.dtype)
    stride_bytes_256 = exact_div(stride_bytes, 256)
    _in_ap = g.lower_ap_dma(in_ap, for_custom_bir_dma=True)
    _idxs_ap = g.lower_ap(idxs_ap)
    _out_ap = g.lower_ap(out_ap)
    return g.add_instruction(
        mybir.InstDMAGatherAnt(
            name=g.bass.get_next_instruction_name(),
            ins=[*_in_ap, _idxs_ap, g.lower_val_access(g.to_reg(num_idxs))],
            outs=[_out_ap],
            transpose=False,
            num_idxs=num_idxs,
            elem_size=elem_size,
            stride_bytes_256=stride_bytes_256,
            gen_mode=0,
            single_packet=False,
            queue_num=queue_num,
            sbuf_tokens_per_rank=0,
            sbuf_free_dim_per_rank=0,
            sbuf_free_dim_pad_per_rank=0,
            sbuf_byte_offset=0,
        ))


def _build_gat(sq, s_alloc):
    """sq: [N_TILES][4] padded slots per quarter (unified over cores);
    s_alloc: [N_TILES] staging slots (sum(sq[t]) rounded up to %8)."""
    import os
    phases = os.environ.get('KV3_PHASES', 'all')
    _build_gat.rr = 0
    s_cap = max(s_alloc)
    stot = [sum(sq[t]) for t in range(N_TILES)]
    ioff = np.concatenate([[0], np.cumsum([8 * s for s in stot])])
    moff = np.concatenate([[0], np.cumsum(s_alloc)])

    nc = bacc.Bacc("TRN2", target_bir_lowering=False, debug=False,
                   num_devices=N_CORES, dynamic_dma_scratch_size=32768,
                   num_swdge_queues=4)
    xeT = nc.dram_tensor("xeT", [128, N_TILES * NI * 128], F16,
                         kind="ExternalInput").ap()
    rhs1 = nc.dram_tensor("rhs1", [128, R1], F16, kind="ExternalInput").ap()
    rhs2 = nc.dram_tensor("rhs2", [H1, 144], F16, kind="ExternalInput").ap()
    idx16 = nc.dram_tensor("idx16", [128, int(ioff[-1])], mybir.dt.int16,
                           kind="ExternalInput").ap()
    maskd = nc.dram_tensor("maskd", [128, int(moff[-1])], F16,
                           kind="ExternalInput").ap()
    selfid = nc.dram_tensor("selfid", [128, N_TILES], mybir.dt.int32,
                            kind="ExternalInput").ap()
    out = nc.dram_tensor("out", [S_CORE, F2], FP, kind="ExternalOutput").ap()

    table2 = nc.dram_tensor("table2", [NP, R2S], F16).ap()
    x2T_shard = nc.dram_tensor("x2T_shard", [H1, SP], F16).ap()
    x2T_all = [nc.dram_tensor(f"x2T_all{p}", [N_CORES * H1, 128 * AG_TILES[p]],
                              F16, addr_space="Shared").ap()
               for p in range(4)]
    x2T_bounce = [nc.dram_tensor(f"x2T_bounce{p}", [H1, 128 * AG_TILES[p]],
                                 F16).ap()
                  for p in range(4)]

    with TileContext(nc) as tc:
        with tc.tile_pool(name="const", bufs=1) as cpool, \
             tc.tile_pool(name="a1", bufs=2) as a1p, \
             tc.tile_pool(name="tb", bufs=2) as tbp, \
             tc.tile_pool(name="a2", bufs=2) as a2p, \
             tc.tile_pool(name="ps_mm", bufs=4, space="PSUM") as pp, \
             tc.tile_pool(name="ps_tr", bufs=2, space="PSUM") as pp_tr, \
             tc.tile_pool(name="ps_t2", bufs=2, space="PSUM") as pp_t2:

            rt1 = cpool.tile([128, R1], F16)
            nc.sync.dma_start(out=rt1[:], in_=rhs1[:, :])
            rt2 = cpool.tile([H1, 144], F16)
            nc.sync.dma_start(out=rt2[:], in_=rhs2[:, :])
            ident = cpool.tile([128, 128], FP)
            make_identity(nc, ident[:])
            mall = cpool.tile([128, int(moff[-1])], F16)
            nc.sync.dma_start(out=mall[:], in_=maskd[:, :])
            sall = cpool.tile([128, N_TILES], mybir.dt.int32)
            nc.sync.dma_start(out=sall[:], in_=selfid[:, :])

            # ---- phase A1: layer-1 attention, host-expanded edges ----
            for t in range(N_TILES):
                r0 = t * 128
                M = min(128, S_CORE - r0)
                lt = a1p.tile([128, NI * 128], F16, name=f"a1l{t}", tag="a1l")
                nc.sync.dma_start(
                    out=lt[:], in_=xeT[:, t * NI * 128:(t + 1) * NI * 128])
                hg = a1p.tile([128, NI * R1], F16, name=f"a1g{t}", tag="a1g")
                hgv = hg[:].rearrange("p (n r) -> p n r", r=R1)
                for gi, j0 in enumerate(range(0, NI, 6)):
                    n6 = min(6, NI - j0)
                    ps = pp.tile([128, n6 * R1], FP, name=f"a1p{t}_{j0}",
                                 tag="a1p", space="PSUM")
                    for j in range(j0, j0 + n6):
                        nc.tensor.matmul(out=ps[:, (j - j0) * R1:
                                                 (j - j0 + 1) * R1],
                                         lhsT=lt[:, j * 128:(j + 1) * 128],
                                         rhs=rt1[:], start=True, stop=True)
                    if gi % 3 == 0:
                        nc.vector.tensor_copy(
                            out=hg[:, j0 * R1:(j0 + n6) * R1], in_=ps[:])
                    else:
                        nc.scalar.activation(
                            out=hg[:, j0 * R1:(j0 + n6) * R1], in_=ps[:],
                            func=AF.Copy)
                u = a1p.tile([128, D_NBR * K1], F16, name=f"a1u{t}", tag="a1u")
                nc.vector.tensor_tensor(
                    out=u[:].rearrange("p (d k) -> p d k", k=K1),
                    in0=hgv[:, 1:, H1 + 8:H1 + 16],
                    in1=hgv[:, 0:1, H1:H1 + 8].to_broadcast([128, D_NBR, K1]),
                    op=OP.add)
                nc.scalar.activation(out=u[:], in_=u[:], func=AF.Prelu,
                                     alpha=NEG_SLOPE)
                nc.scalar.activation(out=u[:], in_=u[:], func=AF.Exp)
                v = a1p.tile([128, D_NBR * Q1], F16, name=f"a1v{t}", tag="a1v")
                vv = v[:].rearrange("p (d q) -> p d q", q=Q1)
                u3d = u[:].rearrange("p (d k) -> p d k", k=K1)
                nc.vector.tensor_tensor(
                    out=vv[:, :, 0:H1].rearrange("p d (k f) -> p d k f", f=F1),
                    in0=hgv[:, 1:, 0:H1].rearrange("p d (k f) -> p d k f",
                                                   f=F1),
                    in1=u3d.unsqueeze(3).to_broadcast([128, D_NBR, K1, F1]),
                    op=OP.mult)
                nc.vector.tensor_copy(out=vv[:, :, H1:Q1], in_=u3d)
                w1 = a1p.tile([128, 16 * Q1], F16, name=f"a1w1{t}", tag="a1w1")
                nc.vector.tensor_tensor(out=w1[:], in0=v[:, :16 * Q1],
                                        in1=v[:, 16 * Q1:32 * Q1], op=OP.add)
                w2 = a1p.tile([128, 8 * Q1], F16, name=f"a1w2{t}", tag="a1w2")
                nc.vector.tensor_tensor(out=w2[:], in0=w1[:, :8 * Q1],
                                        in1=w1[:, 8 * Q1:16 * Q1], op=OP.add)
                w3 = a1p.tile([128, 4 * Q1], F16, name=f"a1w3{t}", tag="a1w3")
                nc.vector.tensor_tensor(out=w3[:], in0=w2[:, :4 * Q1],
                                        in1=w2[:, 4 * Q1:8 * Q1], op=OP.add)
                w4 = a1p.tile([128, 2 * Q1], F16, name=f"a1w4{t}", tag="a1w4")
                nc.vector.tensor_tensor(out=w4[:], in0=w3[:, :2 * Q1],
                                        in1=w3[:, 2 * Q1:4 * Q1], op=OP.add)
                st = a1p.tile([128, Q1], FP, name=f"a1s{t}", tag="a1s")
                nc.vector.tensor_tensor(out=st[:], in0=w4[:, :Q1],
                                        in1=w4[:, Q1:2 * Q1], op=OP.add)
                rz = a1p.tile([128, K1], FP, name=f"a1rz{t}", tag="a1rz")
                nc.vector.reciprocal_approx_fast(out=rz[:], in_=st[:, H1:Q1])
                o = a1p.tile([128, H1], FP, name=f"a1o{t}", tag="a1o")
                nc.vector.tensor_tensor(
                    out=o[:].rearrange("p (k f) -> p k f", f=F1),
                    in0=st[:, 0:H1].rearrange("p (k f) -> p k f", f=F1),
                    in1=rz[:].unsqueeze(2).to_broadcast([128, K1, F1]),
                    op=OP.mult)
                mn = a1p.tile([128, H1], FP, name=f"a1m{t}", tag="a1m")
                nc.vector.tensor_scalar_min(out=mn[:], in0=o[:], scalar1=0.0)
                nc.scalar.activation(out=mn[:], in_=mn[:], func=AF.Exp)
                x2 = a1p.tile([128, H1], FP, name=f"a1x{t}", tag="a1x")
                nc.vector.scalar_tensor_tensor(
                    out=x2[:], in0=mn[:], scalar=-1.0, in1=o[:],
                    op0=OP.add, op1=OP.max)
                pt = pp_tr.tile([H1, 128], FP, name=f"a1pt{t}",
                                tag="a1pt", space="PSUM")
                nc.tensor.transpose(out=pt[:], in_=x2[:], identity=ident[:])
                xt = a1p.tile([H1, 128], F16, name=f"a1xt{t}", tag="a1xt")
                nc.vector.tensor_copy(out=xt[:], in_=pt[:])
                nc.sync.dma_start(out=x2T_shard[:, r0:r0 + 128], in_=xt[:])

                # pipelined AllGather: ship finished column parts during A1
                if phases != 'a1' and t + 1 in (25, 50, 75, 98):
                    p = (25, 50, 75, 98).index(t + 1)
                    c0, w = AG_OFF[p] * 128, AG_TILES[p] * 128
                    nc.sync.dma_start(out=x2T_bounce[p][:, :],
                                      in_=x2T_shard[:, c0:c0 + w])
                    nc.gpsimd.collective_compute(
                        "AllGather", OP.bypass,
                        replica_groups=[list(range(N_CORES))],
                        ins=[x2T_bounce[p].opt()], outs=[x2T_all[p].opt()])

            # ---- phase T2: table2 rows [s1|h2|s2] @ 512B stride ----
            if phases == 'a1':
                zz = a1p.tile([128, F2], FP, name="zz", tag="zz")
                nc.vector.memset(zz[:], 0.0)
                nc.sync.dma_start(out=out[0:128, :], in_=zz[:])

            def t2_part(c):
                """global chunk -> (core r, part p, col within part)."""
                r, lc = divmod(c, N_TILES)
                p = min(lc // 25, 3)
                return r, p, (lc - AG_OFF[p]) * 128

            def t2_load(lt2, g0c, nck):
                """Load chunks [g0c, g0c+nck) merging runs within one
                (core, part) block."""
                s = 0
                while s < nck:
                    r, p, col = t2_part(g0c + s)
                    e = s + 1
                    while e < nck:
                        r2, p2, col2 = t2_part(g0c + e)
                        if (r2, p2) != (r, p) or col2 != col + (e - s) * 128:
                            break
                        e += 1
                    nc.sync.dma_start(
                        out=lt2[:, s * 128:e * 128],
                        in_=x2T_all[p][r * H1:(r + 1) * H1,
                                       col:col + (e - s) * 128])
                    s = e

            n_grp2 = (N_CHUNKS // TG) if phases != 'a1' else 0
            for g in range(n_grp2):
                g0 = g * TG * 128
                lt2 = tbp.tile([H1, TG * 128], F16, name=f"t2l{g}", tag="t2l")
                t2_load(lt2, g * TG, TG)
                rows2 = tbp.tile([128, TG * 144], F16, name=f"t2r{g}",
                                 tag="t2r")
                for cc in range(0, TG, 2):
                    ps2 = pp_t2.tile([128, 288], FP, name=f"t2p{g}_{cc}",
                                     tag="t2p", space="PSUM")
                    for k in range(2):
                        nc.tensor.matmul(
                            out=ps2[:, k * 144:(k + 1) * 144],
                            lhsT=lt2[:, (cc + k) * 128:(cc + k + 1) * 128],
                            rhs=rt2[:], start=True, stop=True)
                    if cc % 4 == 0:
                        nc.vector.tensor_copy(
                            out=rows2[:, cc * 144:(cc + 2) * 144], in_=ps2[:])
                    else:
                        nc.scalar.activation(
                            out=rows2[:, cc * 144:(cc + 2) * 144], in_=ps2[:],
                            func=AF.Copy)
                nc.sync.dma_start(
                    out=table2[g0:g0 + TG * 128, 0:144]
                        .rearrange("(c p) r -> p c r", p=128),
                    in_=rows2[:].rearrange("p (c r) -> p c r", r=144))

            # ---- phase A2: quarter-split gathers + attention ----
            if phases == 't2':
                zz = a1p.tile([128, F2], FP, name="zz2", tag="zz")
                nc.vector.memset(zz[:], 0.0)
                nc.sync.dma_start(out=out[0:128, :], in_=zz[:])
            for t in range(N_TILES if phases == 'all' else 0):
                r0 = t * 128
                M = min(128, S_CORE - r0)
                SQt, SA = sq[t], s_alloc[t]
                ST = sum(SQt)
                it2 = a2p.tile([128, 8 * s_cap], mybir.dt.int16,
                               name=f"a2i{t}", tag="a2i")
                nc.sync.dma_start(out=it2[:, :8 * ST],
                                  in_=idx16[:, int(ioff[t]):int(ioff[t]) +
                                            8 * ST])
                hg2 = a2p.tile([128, s_cap * GE2], F16, name=f"a2g{t}",
                               tag="a2g")
                if t < 2:
                    nc.vector.memset(hg2[:], 0.0)
                hg2v = hg2[:].rearrange("p (s e) -> p s e", e=GE2)
                off = 0
                xoff = 0
                for q in range(4):
                    for c0 in range(0, SQt[q], 15):
                        sub = min(15, SQt[q] - c0)
                        _gather_raw(
                            nc,
                            out_ap=hg2v[:, off + c0:off + c0 + sub, :],
                            in_ap=table2[q * QN:(q + 1) * QN, 8:8 + GE2],
                            idxs_ap=it2[:, xoff + 8 * c0:xoff + 8 * (c0 + sub)],
                            num_idxs=128 * sub, elem_size=GE2, elem_step=R2S,
                            queue_num=_build_gat.rr % 4)
                        _build_gat.rr += 1
                    off += SQt[q]
                    xoff += 8 * SQt[q]
                s1d = a2p.tile([128, K2], F16, name=f"a2sd{t}", tag="a2sd")
                nc.gpsimd.indirect_dma_start(
                    out=s1d[:], out_offset=None, in_=table2[:],
                    in_offset=bass.IndirectOffsetOnAxis(
                        ap=sall[:, t:t + 1], axis=0))
                u2 = a2p.tile([128, s_cap * K2], F16, name=f"a2u{t}",
                              tag="a2u")
                u2v = u2[:].rearrange("p (s k) -> p s k", k=K2)
                nc.vector.tensor_tensor(
                    out=u2v[:, :SA, :], in0=hg2v[:, :SA, H2:H2 + 8],
                    in1=s1d[:].unsqueeze(1).to_broadcast([128, SA, K2]),
                    op=OP.add)
                nc.scalar.activation(out=u2[:, :SA * K2], in_=u2[:, :SA * K2],
                                     func=AF.Prelu, alpha=NEG_SLOPE)
                nc.scalar.activation(out=u2[:, :SA * K2], in_=u2[:, :SA * K2],
                                     func=AF.Exp)
                nc.vector.tensor_tensor(
                    out=u2v[:, :SA, :], in0=u2v[:, :SA, :],
                    in1=mall[:, int(moff[t]):int(moff[t]) + SA]
                        .unsqueeze(2).to_broadcast([128, SA, K2]),
                    op=OP.mult)
                v2 = a2p.tile([128, s_cap * Q2], F16, name=f"a2v{t}",
                              tag="a2v")
                vv2 = v2[:].rearrange("p (s q) -> p s q", q=Q2)
                nc.vector.tensor_tensor(
                    out=vv2[:, :SA, 0:H2].rearrange("p s (k f) -> p s k f",
                                                    f=F2),
                    in0=hg2v[:, :SA, 0:H2].rearrange("p s (k f) -> p s k f",
                                                     f=F2),
                    in1=u2v[:, :SA, :].unsqueeze(3)
                        .to_broadcast([128, SA, K2, F2]),
                    op=OP.mult)
                nc.vector.tensor_copy(out=vv2[:, :SA, H2:Q2],
                                      in_=u2v[:, :SA, :])
                # tree: SA (=8m) -> SA/2 -> SA/4 -> SA/8 = m, then reduce m
                w1 = a2p.tile([128, (s_cap // 2) * Q2], F16, name=f"a2w1{t}",
                              tag="a2w1")
                h = SA // 2
                nc.vector.tensor_tensor(out=w1[:, :h * Q2], in0=v2[:, :h * Q2],
                                        in1=v2[:, h * Q2:2 * h * Q2],
                                        op=OP.add)
                h2_ = SA // 4
                nc.vector.tensor_tensor(out=v2[:, :h2_ * Q2],
                                        in0=w1[:, :h2_ * Q2],
                                        in1=w1[:, h2_ * Q2:2 * h2_ * Q2],
                                        op=OP.add)
                m = SA // 8
                nc.vector.tensor_tensor(out=w1[:, :m * Q2],
                                        in0=v2[:, :m * Q2],
                                        in1=v2[:, m * Q2:2 * m * Q2],
                                        op=OP.add)
                st2 = a2p.tile([128, Q2], FP, name=f"a2s{t}", tag="a2s")
                if m == 1:
                    nc.vector.tensor_copy(out=st2[:], in_=w1[:, :Q2])
                else:
                    nc.vector.tensor_reduce(
                        out=st2[:],
                        in_=w1[:, :m * Q2]
                            .rearrange("p (s q) -> p s q", q=Q2)
                            .transpose([0, 2, 1]),
                        axis=AX.X, op=OP.add)
                rz2 = a2p.tile([128, K2], FP, name=f"a2rz{t}", tag="a2rz")
                nc.vector.reciprocal_approx_fast(out=rz2[:], in_=st2[:, H2:Q2])
                o2 = a2p.tile([128, H2], FP, name=f"a2o{t}", tag="a2o")
                nc.vector.tensor_tensor(
                    out=o2[:].rearrange("p (k f) -> p k f", f=F2),
                    in0=st2[:, 0:H2].rearrange("p (k f) -> p k f", f=F2),
                    in1=rz2[:].unsqueeze(2).to_broadcast([128, K2, F2]),
                    op=OP.mult)
                mo = a2p.tile([128, F2], FP, name=f"a2mo{t}", tag="a2mo")
                nc.vector.tensor_reduce(
                    out=mo[:],
                    in_=o2[:].rearrange("p (k f) -> p k f", f=F2)
                        .transpose([0, 2, 1]),
                    axis=AX.X, op=OP.add)
                u3 = a2p.tile([128, F2], FP, name=f"a2u3{t}", tag="a2u3")
                z3 = a2p.tile([128, 1], FP, name=f"a2z3{t}", tag="a2z3")
                nc.scalar.activation(out=u3[:], in_=mo[:], func=AF.Exp,
                                     scale=1.0 / K2, accum_out=z3[:])
                rz3 = a2p.tile([128, 1], FP, name=f"a2rz3{t}", tag="a2rz3")
                nc.vector.reciprocal_approx_fast(out=rz3[:], in_=z3[:])
                ot = a2p.tile([128, F2], FP, name=f"a2ot{t}", tag="a2ot")
                nc.vector.tensor_tensor(
                    out=ot[:], in0=u3[:],
                    in1=rz3[:].to_broadcast([128, F2]), op=OP.mult)
                nc.sync.dma_start(out=out[r0:r0 + M, :], in_=ot[:M, :])

    nc.finalize()
    return nc


class _SpmdRunner:
    """jit-once SPMD executor over the 8 axon NeuronCores."""

    def __init__(self, nc, n_cores):
        install_neuronx_cc_hook()
        self.nc, self.n_cores = nc, n_cores
        partition_name = (nc.partition_id_tensor.name
                          if nc.partition_id_tensor else None)
        in_names, out_names, out_avals, zero_outs = [], [], [], []
        for alloc in nc.m.functions[0].allocations:
            if not isinstance(alloc, mybir.MemoryLocationSet):
                continue
            name = alloc.memorylocations[0].name
            if alloc.kind == "ExternalInput":
                if name != partition_name:
                    in_names.append(name)
            elif alloc.kind == "ExternalOutput":
                out_names.append(name)
                shape = tuple(alloc.tensor_shape)
                dtype = mybir.dt.np(alloc.dtype)
                out_avals.append(jax.core.ShapedArray(shape, dtype))
                zero_outs.append(np.zeros(shape, dtype))
        self.in_names, self.out_names = in_names, out_names
        self.out_avals, self.zero_outs = out_avals, zero_outs
        all_in_names = in_names + out_names
        if partition_name is not None:
            all_in_names.append(partition_name)

        def _body(*args):
            operands = list(args)
            if partition_name is not None:
                operands.append(partition_id_tensor())
            return tuple(_bass_exec_p.bind(
                *operands, out_avals=tuple(out_avals),
                in_names=tuple(all_in_names), out_names=tuple(out_names),
                lowering_input_output_aliases=(),
                sim_require_finite=True, sim_require_nnan=True, nc=nc))

        devices = jax.devices()[:n_cores]
        self.mesh = Mesh(np.asarray(devices), ("core",))
        n_params, n_outs = len(in_names), len(out_avals)
        in_specs = (PartitionSpec("core"),) * (n_params + n_outs)
        out_specs = (PartitionSpec("core"),) * n_outs
        self.fn = jax.jit(
            shard_map(_body, mesh=self.mesh, in_specs=in_specs,
                      out_specs=out_specs, check_rep=False),
            keep_unused=True)
        self.sharding = jax.sharding.NamedSharding(self.mesh,
                                                   PartitionSpec("core"))

    def run(self, in_maps):
        per_core = [[np.asarray(m[n]) for n in self.in_names] for m in in_maps]
        concat = [np.concatenate([per_core[c][i] for c in range(self.n_cores)],
                                 axis=0) for i in range(len(self.in_names))]
        zeros = [np.zeros((self.n_cores * z.shape[0], *z.shape[1:]), z.dtype)
                 for z in self.zero_outs]
        dev = [jax.device_put(a, self.sharding) for a in concat + zeros]
        outs = self.fn(*dev)
        jax.block_until_ready(outs)
        res = []
        for c in range(self.n_cores):
            res.append({name: np.asarray(outs[i]).reshape(
                self.n_cores, *self.out_avals[i].shape)[c]
                for i, name in enumerate(self.out_names)})
        return res


def _wrap16(lst):
    """[n] -> [128, n//16]: wrapped in 16 partitions, replicated x8 groups."""
    n = len(lst)
    w = np.asarray(lst, np.int16).reshape(n // 16, 16).T  # [16, n//16]
    return np.tile(w, (8, 1))


def _plan(neighbors):
    """Data-dependent plan: per-core permutation + unified per-tile quarter
    widths + per-core idx16/mask/selfid arrays."""
    nbr0 = np.asarray(neighbors, np.int64).reshape(N_CORES, S_CORE, D_NBR)
    # padded table ids: node (core c, local i) -> c*SP + i
    nbr = (nbr0 // S_CORE) * SP + nbr0 % S_CORE
    qq = (nbr // QN).astype(np.int32)                     # [8, S, 32]
    counts = np.zeros((N_CORES, S_CORE, 4), np.int32)
    for q in range(4):
        counts[:, :, q] = (qq == q).sum(axis=2)
    perms = []
    percore_cnt = []
    for r in range(N_CORES):
        key = counts[r].max(axis=1)
        perm = np.argsort(-key, kind='stable').astype(np.int32)
        perm_pad = np.concatenate(
            [perm, np.full(SP - S_CORE, perm[-1], np.int32)])
        perms.append(perm_pad)
        percore_cnt.append(counts[r][perm_pad])           # [SP, 4]
    # unified per-tile quarter widths
    sq = []
    for t in range(N_TILES):
        mx = np.zeros(4, np.int64)
        for r in range(N_CORES):
            mx = np.maximum(mx, percore_cnt[r][t * 128:(t + 1) * 128]
                            .max(axis=0))
        sq.append([int(x) for x in mx])
    s_alloc = [max(8, -(-sum(s) // 8) * 8) for s in sq]

    idx16s, masks, selfids = [], [], []
    for r in range(N_CORES):
        perm_pad = perms[r]
        srt = np.sort(nbr[r], axis=1)                     # [S, 32] sorted src
        srt_pad = srt[perm_pad]                           # [SP, 32]
        cnt_pad = percore_cnt[r]                          # [SP, 4]
        start = np.concatenate([np.zeros((SP, 1), np.int64),
                                np.cumsum(cnt_pad, axis=1)[:, :3]], axis=1)
        iblocks, mblocks = [], []
        for t in range(N_TILES):
            rows = slice(t * 128, (t + 1) * 128)
            mtile = np.zeros((128, s_alloc[t]), np.float16)
            off = 0
            for q in range(4):
                SQ = sq[t][q]
                if SQ == 0:
                    continue
                k = np.arange(SQ)[None, :]
                gidx = start[rows, q:q + 1] + k
                valid = k < cnt_pad[rows, q:q + 1]
                vals = np.take_along_axis(
                    srt_pad[rows], np.minimum(gidx, D_NBR - 1), axis=1)
                vals = np.where(valid, vals - q * QN, 0).astype(np.int16)
                iblocks.append(_wrap16(vals.T.ravel()))
                mtile[:, off:off + SQ] = valid.astype(np.float16)
                off += SQ
            mblocks.append(mtile)
        idx16s.append(np.concatenate(iblocks, axis=1))
        masks.append(np.concatenate(mblocks, axis=1))
        gid = (r * SP + perm_pad).astype(np.int32)        # [SP] padded ids
        selfids.append(np.ascontiguousarray(
            gid.reshape(N_TILES, 128).T))                 # [128, N_TILES]
    return sq, s_alloc, perms, idx16s, masks, selfids


def _host_prep(node_features, neighbors, W1, a1_1, a2_1, W2, a1_2, a2_2,
               plan):
    sq, s_alloc, perms, idx16s, masks, selfids = plan

    def blk(a, k, f):
        A = np.zeros((k * f, k), np.float32)
        for kk in range(k):
            A[kk * f:(kk + 1) * f, kk] = a[kk]
        return A

    rhs1 = np.concatenate(
        [W1, W1 @ blk(a1_1, K1, F1), W1 @ blk(a2_1, K1, F1)],
        axis=1).astype(np.float16)                        # [128, 80] h|s1|s2
    rhs2 = np.concatenate(
        [W2 @ blk(a1_2, K2, F2), W2, W2 @ blk(a2_2, K2, F2)],
        axis=1).astype(np.float16)                        # [64, 144] s1|h2|s2
    xT = node_features.T.astype(np.float16)               # [128, N]
    nbr = np.asarray(neighbors, np.int64)

    in_maps = []
    for r in range(N_CORES):
        ids = np.arange(r * S_CORE, (r + 1) * S_CORE, dtype=np.int64)
        nb33 = np.concatenate([ids[:, None], nbr[r * S_CORE:(r + 1) * S_CORE]],
                              axis=1)                     # [S, 33]
        nb33 = np.concatenate(
            [nb33, np.zeros((SP - S_CORE, NI), np.int64)], axis=0)
        # xeT[:, t*NI*128 + j*128 + p] = xT[:, nb33[t*128+p, j]]
        flat = nb33.reshape(N_TILES, 128, NI).transpose(0, 2, 1).ravel()
        xeT = np.ascontiguousarray(xT[:, flat])
        in_maps.append({'xeT': xeT, 'rhs1': rhs1, 'rhs2': rhs2,
                        'idx16': idx16s[r], 'maskd': masks[r],
                        'selfid': selfids[r]})
    return in_maps


_RUNNER = None
_PLAN = None
_PLAN_KEY = None


def _get_runner(neighbors):
    global _RUNNER, _PLAN, _PLAN_KEY
    key = hash(np.asarray(neighbors, np.int64).tobytes())
    if _RUNNER is None or key != _PLAN_KEY:
        _PLAN = _plan(neighbors)
        _PLAN_KEY = key
        nc = _build_gat(_PLAN[0], _PLAN[1])
        _RUNNER = _SpmdRunner(nc, N_CORES)
    return _RUNNER, _PLAN


def kernel(node_features, neighbors, W1, a1_1, a2_1, W2, a1_2, a2_2):
    node_features = np.asarray(node_features, dtype=np.float32)
    neighbors = np.asarray(neighbors)
    runner, plan = _get_runner(neighbors)
    in_maps = _host_prep(node_features, neighbors,
                         np.asarray(W1, np.float32),
                         np.asarray(a1_1, np.float32),
                         np.asarray(a2_1, np.float32),
                         np.asarray(W2, np.float32),
                         np.asarray(a1_2, np.float32),
                         np.asarray(a2_2, np.float32), plan)
    res = runner.run(in_maps)
    out = np.empty((N_NODES, F2), np.float32)
    for r in range(N_CORES):
        perm = plan[2][r][:S_CORE]
        out[r * S_CORE + perm] = res[r]['out']
    return out

